# revision 36
# baseline (speedup 1.0000x reference)
"""Trainium2 Bass kernel for BipartiteGCN (8 NeuronCores, SPMD).

Strategy:
 - Node rows sharded 8 ways (cons: NC/8 rows per core, var: NV/8).
 - Edges sharded by DESTINATION range; per-core edges sorted by dst block
   (128 dst rows per block), bucketed by src>=32768 where needed (int16
   gather indices).
 - Per-edge pipeline: dma_gather of lp[src] and rp[dst] rows (bf16),
   add -> LayerNorm (bn_stats) -> LeakyReLU fused on ScalarE -> one-hot
   (iota is_equal) -> matmul-accumulate into PSUM per dst block: computes
   segment-sum AND counts (ones column) with no scatter DMA.
 - Linearity: wf/bias applied after the segment-mean (per node, not per
   edge).
 - Only lp tables are all-gathered; rp/skip/post-MLP/head stages stay
   local to each core's dst shard. Output is the var shard -> host concat.
"""

import os
import sys

for _p in ("/opt/trn_rl_repo",):
    if _p not in sys.path:
        sys.path.insert(0, _p)

import numpy as np
import ml_dtypes

import concourse.bass as bass
import concourse.bacc as bacc
import concourse.mybir as mybir
from concourse import tile, library_config
from concourse.bass_utils import run_bass_kernel_spmd
from concourse import hw_specs as _hw_specs

# The stock 0.34 ns/descriptor SWDGE estimate is calibrated on plain SWDGE
# dma_start; the Q7 dma_gather firmware measures ~7.7 ns/descriptor. With the
# low estimate the Tile scheduler orders consumers of a just-issued gather
# ahead of ready work, causing head-of-line blocking in engine FIFOs.
_hw_specs.TRN2Spec.SWDGE_NS_PER_DESCRIPTOR = 7.7

BF16 = ml_dtypes.bfloat16
F32 = np.float32
NCORES = 8
EMB = 128
CHUNK_TILES = 32     # tiles (128 edges) per dma_gather call (4096 edges)
HI_BASE = 32768
EPS = 1e-5
SLOPE = 0.01

dt = mybir.dt


def _wrap_idx(idx_i16):
    """[N] int16 -> [128, N//16] wrapped (i at [i%16, i//16]) + replicated 8x."""
    n = idx_i16.shape[0]
    assert n % 16 == 0
    w = idx_i16.reshape(n // 16, 16).T
    return np.tile(w, (8, 1)).copy()


def _lane_major(arr, lanes=128):
    """[N] -> [lanes, N//lanes] with element i at [i%lanes, i//lanes]."""
    n = arr.shape[0]
    assert n % lanes == 0
    return arr.reshape(n // lanes, lanes).T.copy()


def _bcast_row(v, rows=128):
    """[F] -> [rows, F] replicated, f32."""
    return np.broadcast_to(np.asarray(v, F32)[None, :], (rows, v.shape[0])).copy()


class ConvPrep:
    """Per-conv edge-sharding data. Same segment layout for all cores."""

    def __init__(self, dst, src, n_dst, n_src, dst_per_core):
        self.n_dst_local = dst_per_core
        self.nblocks = -(-dst_per_core // 128)
        self.two_buckets = n_src > HI_BASE
        nb = self.nblocks
        nu = 2 if self.two_buckets else 1

        core = dst // dst_per_core
        dloc_all = dst - core * dst_per_core
        block_all = dloc_all // 128

        # per (core, bucket, block) edge lists
        per = [[[None] * nb for _ in range(nu)] for _ in range(NCORES)]
        for c in range(NCORES):
            m = core == c
            d_c = dloc_all[m]
            s_c = src[m]
            b_c = block_all[m]
            u_c = (s_c >= HI_BASE).astype(np.int8) if self.two_buckets else np.zeros(
                len(s_c), np.int8
            )
            for u in range(nu):
                mu = u_c == u
                db, sb, bb = d_c[mu], s_c[mu], b_c[mu]
                order = np.argsort(bb, kind="stable")
                db, sb, bb = db[order], sb[order], bb[order]
                bounds = np.searchsorted(bb, np.arange(nb + 1))
                for b in range(nb):
                    lo, hi = bounds[b], bounds[b + 1]
                    per[c][u][b] = (sb[lo:hi], db[lo:hi])

        # uniform tile counts (128 edges): max over cores
        self.ntiles = np.zeros((nu, nb), np.int64)
        for u in range(nu):
            for b in range(nb):
                mx = max(len(per[c][u][b][0]) for c in range(NCORES))
                self.ntiles[u, b] = -(-mx // 128) if mx > 0 else 0

        etot = int(self.ntiles.sum()) * 128
        self.etot = etot

        # host-known scatter-mean counts per core: [128, nb] lane-major
        self.recip = np.zeros((NCORES, 128, nb), F32)
        self.ind = np.zeros((NCORES, 128, nb), F32)
        for c in range(NCORES):
            cnt = np.bincount(dloc_all[core == c], minlength=nb * 128).astype(F32)
            cnt = cnt[: nb * 128].reshape(nb, 128).T  # [128, nb]
            self.recip[c] = 1.0 / np.maximum(cnt, 1.0)
            self.ind[c] = np.minimum(cnt, 1.0)

        # build padded per-core arrays in segment order (u-major, b-minor)
        self.src_idx = np.zeros((NCORES, etot), np.int16)
        self.dstrel = np.full((NCORES, etot), -1.0, F32)
        off = 0
        self.seg_offsets = {}
        for u in range(nu):
            for b in range(nb):
                g = int(self.ntiles[u, b])
                if g == 0:
                    continue
                self.seg_offsets[(u, b)] = off
                for c in range(NCORES):
                    sb, db = per[c][u][b]
                    n = len(sb)
                    s_adj = sb - (HI_BASE if u == 1 else 0)
                    self.src_idx[c, off : off + n] = s_adj.astype(np.int16)
                    self.dstrel[c, off : off + n] = (db - 128 * b).astype(F32)
                off += g * 128
        assert off == etot

        # stream layout: per bucket, list of (block, ntiles)
        self.streams = []
        for u in range(nu):
            blocks = [(b, int(self.ntiles[u, b])) for b in range(nb) if self.ntiles[u, b] > 0]
            start = self.seg_offsets[(u, blocks[0][0])] if blocks else 0
            nt = sum(g for _, g in blocks)
            self.streams.append({"u": u, "blocks": blocks, "start_edge": start, "ntiles": nt})

    def core_arrays(self, c):
        dr = self.dstrel[c]
        i = np.nonzero(dr >= 0)[0]
        lane = i % 128
        tb = (i // 128) * 128
        d = dr[i].astype(np.int64)
        oh = np.zeros((128, self.etot), ml_dtypes.float8_e4m3)
        oh[lane, tb + d] = 1.0
        ohT = np.zeros((128, self.etot), ml_dtypes.float8_e4m3)
        ohT[d, tb + lane] = 1.0
        return _wrap_idx(self.src_idx[c]), oh, ohT


def host_prep(inputs):
    p = {}
    cons_x = np.asarray(inputs["cons_x"], F32)
    var_x = np.asarray(inputs["var_x"], F32)
    edge_cons = np.asarray(inputs["edge_cons"]).astype(np.int64)
    edge_var = np.asarray(inputs["edge_var"]).astype(np.int64)
    head_mask = np.asarray(inputs["head_mask"]).astype(bool)

    NC, CF = cons_x.shape
    NV, VF = var_x.shape
    assert NC % NCORES == 0 and NV % NCORES == 0
    NCL, NVL = NC // NCORES, NV // NCORES
    p.update(NC=NC, NV=NV, CF=CF, VF=VF, NCL=NCL, NVL=NVL)

    # conv1: v->c (src=edge_var over NV, dst=edge_cons over NC)
    p["conv1"] = ConvPrep(edge_cons, edge_var, NC, NV, NCL)
    # conv2: c->v
    p["conv2"] = ConvPrep(edge_var, edge_cons, NV, NC, NVL)

    # ---- weights ----
    w = {}

    def embed_w(prefix, g, b, w1, b1, w2, b2, feat):
        w1 = np.asarray(w1, F32)
        w1g = np.asarray(g, F32)[:, None] * w1
        aug = np.concatenate([w1g, np.zeros((1, w1.shape[1]), F32)], 0)
        w[prefix + "w1aug"] = aug.astype(BF16)
        w[prefix + "s1"] = _bcast_row(np.asarray(b, F32) @ w1 + np.asarray(b1, F32))
        w[prefix + "r1"] = _bcast_row(w1g.sum(0))
        w[prefix + "w2"] = np.asarray(w2, F32).astype(BF16)
        w[prefix + "b2"] = _bcast_row(np.asarray(b2, F32))

    embed_w("ce_", inputs["ce_ln_g"], inputs["ce_ln_b"], inputs["ce_w1"],
            inputs["ce_b1"], inputs["ce_w2"], inputs["ce_b2"], CF)
    embed_w("ve_", inputs["ve_ln_g"], inputs["ve_ln_b"], inputs["ve_w1"],
            inputs["ve_b1"], inputs["ve_w2"], inputs["ve_b2"], VF)

    for pre in ("vc_", "cv_"):
        wl = np.asarray(inputs[pre + "wl"], F32)
        w[pre + "wl"] = wl.astype(BF16)
        w[pre + "bl"] = _bcast_row(np.asarray(inputs[pre + "bl"], F32))
        w[pre + "wr"] = np.asarray(inputs[pre + "wr"], F32).astype(BF16)
        flg = np.asarray(inputs[pre + "flg"], F32)
        flb = np.asarray(inputs[pre + "flb"], F32)
        p[pre + "fl_trivial"] = bool(np.all(flg == 1.0) and np.all(flb == 0.0))
        w[pre + "flg"] = _bcast_row(flg)
        w[pre + "flb"] = _bcast_row(flb)
        w[pre + "wf"] = np.asarray(inputs[pre + "wf"], F32).astype(BF16)
        w[pre + "bf"] = _bcast_row(np.asarray(inputs[pre + "bf"], F32))
        wo1 = np.asarray(inputs[pre + "wo1"], F32)
        plg = np.asarray(inputs[pre + "plg"], F32)
        plb = np.asarray(inputs[pre + "plb"], F32)
        w[pre + "wo1a"] = (plg[:, None] * wo1[:EMB]).astype(BF16)
        w[pre + "wo1b"] = wo1[EMB:].astype(BF16)
        w[pre + "bo1"] = _bcast_row(np.asarray(inputs[pre + "bo1"], F32) + plb @ wo1[:EMB])
        w[pre + "wo2"] = np.asarray(inputs[pre + "wo2"], F32).astype(BF16)
        w[pre + "bo2"] = _bcast_row(np.asarray(inputs[pre + "bo2"], F32))

    # heads
    active = np.nonzero(head_mask)[0]
    nact = int(len(active))
    p["nact"] = nact
    denom = max(float(head_mask.sum()), 1.0)
    hb2 = np.asarray(inputs["hb2"], F32)
    p["out_scale"] = 1.0 / denom
    p["out_add"] = float(hb2[active].sum() / denom)
    if nact > 0:
        hw1 = np.asarray(inputs["hw1"], F32)[active]          # [nact,128,128]
        w["hw1"] = hw1.transpose(1, 0, 2).astype(BF16).copy()  # [128,nact,128]
        w["hb1"] = np.asarray(inputs["hb1"], F32)[active].T.copy()   # [128,nact]
        w["hw2"] = np.asarray(inputs["hw2"], F32)[active].T.astype(BF16).copy()  # [128,nact]

    w["identity"] = np.eye(128, dtype=BF16)
    p["weights"] = w

    # ---- per-core inputs ----
    NCLp = -(-NCL // 128) * 128
    NVLp = -(-NVL // 128) * 128
    p.update(NCLp=NCLp, NVLp=NVLp)
    NVLh = -(-NVL // 512) * 512  # head stage col padding
    p["NVLh"] = NVLh

    core_inputs = []
    for c in range(NCORES):
        m = {}
        cx = cons_x[c * NCL : (c + 1) * NCL]
        vx = var_x[c * NVL : (c + 1) * NVL]
        cxp = np.zeros((NCLp, CF), F32)
        cxp[:NCL] = cx
        vxp = np.zeros((NVLp, VF), F32)
        vxp[:NVL] = vx
        m["cons_rows"] = cxp.reshape(NCLp // 128, 128, CF).transpose(1, 0, 2).copy()
        m["var_rows"] = vxp.reshape(NVLp // 128, 128, VF).transpose(1, 0, 2).copy()
        m["consT_aug"] = np.concatenate([cxp.T, np.ones((1, NCLp), F32)], 0).astype(BF16)
        m["varT_aug"] = np.concatenate([vxp.T, np.ones((1, NVLp), F32)], 0).astype(BF16)
        s1, oh1, ohT1 = p["conv1"].core_arrays(c)
        m["e1_src"], m["e1_oh"], m["e1_ohT"] = s1, oh1, ohT1
        s2, oh2, ohT2 = p["conv2"].core_arrays(c)
        m["e2_src"], m["e2_oh"], m["e2_ohT"] = s2, oh2, ohT2
        m["e1_recip"] = p["conv1"].recip[c]
        m["e1_ind"] = p["conv1"].ind[c]
        m["e2_recip"] = p["conv2"].recip[c]
        m["e2_ind"] = p["conv2"].ind[c]
        for k, v in w.items():
            m[k] = v
        core_inputs.append(m)
    p["core_inputs"] = core_inputs
    return p


# ---------------------------------------------------------------------------
# program builder
# ---------------------------------------------------------------------------


class B:
    """Builder context."""

    def __init__(self, p):
        self.p = p
        self.nc = bacc.Bacc("TRN2", target_bir_lowering=False, debug=False,
                            num_devices=NCORES, num_swdge_queues=2)
        self.d = {}  # dram tensors

    def dram(self, name, shape, dtype, kind=None, addr_space=None):
        kw = {}
        if kind:
            kw["kind"] = kind
        if addr_space:
            kw["addr_space"] = addr_space
        t = self.nc.dram_tensor(name, list(shape), dtype, **kw)
        self.d[name] = t
        return t


MAGIC = 0x5F3759DF


def rsqrt_newton(nc, pool, src_ap, n, tag, iters=2):
    """1/sqrt(src) on DVE only. src_ap [128, n] f32 > 0."""
    AL = mybir.AluOpType
    sh = pool.tile([128, n], dt.int32, tag=tag + "sh")
    nc.vector.tensor_scalar(sh[:], src_ap.bitcast(dt.int32), 1, None,
                            AL.arith_shift_right)
    y0 = pool.tile([128, n], dt.int32, tag=tag + "y0")
    nc.vector.tensor_scalar(y0[:], sh[:], -1, MAGIC, AL.mult, AL.add)
    cur = y0[:].bitcast(dt.float32)
    h = pool.tile([128, n], dt.float32, tag=tag + "h")
    nc.vector.tensor_scalar_mul(h[:], src_ap, 0.5)
    yy = pool.tile([128, n], dt.float32, tag=tag + "yy")
    for it in range(iters):
        nc.vector.tensor_tensor(yy[:], cur, cur, AL.mult)
        nc.vector.tensor_tensor(yy[:], yy[:], h[:], AL.mult)
        nc.vector.tensor_scalar(yy[:], yy[:], -1.0, 1.5, AL.mult, AL.add)
        nxt = pool.tile([128, n], dt.float32, tag=tag + f"n{it}")
        nc.vector.tensor_tensor(nxt[:], cur, yy[:], AL.mult)
        cur = nxt[:]
    return cur


def ln_rows_stats(nc, pool, x_ap, nrows, nfeat):
    """LN stats for rows-major f32 [nrows, nfeat] -> (rstd_ap, nmr, mu). DVE only."""
    s1 = pool.tile([128, 1], dt.float32, tag="s1")
    nc.vector.reduce_sum(s1[:nrows], x_ap, axis=mybir.AxisListType.X)
    sq = pool.tile([128, nfeat], dt.float32, tag="sqscratch")
    s2 = pool.tile([128, 1], dt.float32, tag="s2")
    nc.vector.scalar_tensor_tensor(
        sq[:nrows], x_ap, 0.0, x_ap, mybir.AluOpType.add, mybir.AluOpType.mult,
        accum_out=s2[:nrows],
    )
    inv = 1.0 / nfeat
    musq = pool.tile([128, 1], dt.float32, tag="musq")
    nc.vector.scalar_tensor_tensor(
        musq[:nrows], s1[:nrows], inv * inv, s1[:nrows],
        mybir.AluOpType.mult, mybir.AluOpType.mult,
    )
    veps0 = pool.tile([128, 1], dt.float32, tag="veps0")
    nc.vector.tensor_scalar(veps0[:nrows], s2[:nrows], inv, EPS,
                            mybir.AluOpType.mult, mybir.AluOpType.add)
    veps = pool.tile([128, 1], dt.float32, tag="veps")
    nc.vector.tensor_tensor(veps[:nrows], veps0[:nrows], musq[:nrows],
                            mybir.AluOpType.subtract)
    rstd = rsqrt_newton(nc, pool, veps[:nrows], 1, "lnr")
    mu = pool.tile([128, 1], dt.float32, tag="mu")
    nc.vector.tensor_scalar_mul(mu[:nrows], s1[:nrows], inv)
    nmr = pool.tile([128, 1], dt.float32, tag="nmr")
    nc.vector.scalar_tensor_tensor(
        nmr[:nrows], mu[:nrows], -1.0, rstd,
        mybir.AluOpType.mult, mybir.AluOpType.mult,
    )
    return rstd, nmr, mu


def build_program(p):
    b = B(p)
    nc = b.nc
    w = p["weights"]
    NCL, NVL, NCLp, NVLp = p["NCL"], p["NVL"], p["NCLp"], p["NVLp"]
    CF, VF = p["CF"], p["VF"]
    NC, NV = p["NC"], p["NV"]
    NVLh = p["NVLh"]
    nact = p["nact"]

    # ---- dram declarations ----
    din = lambda n, s, t: b.dram(n, s, t, kind="ExternalInput")
    din("cons_rows", [128, NCLp // 128, CF], dt.float32)
    din("var_rows", [128, NVLp // 128, VF], dt.float32)
    din("consT_aug", [CF + 1, NCLp], dt.bfloat16)
    din("varT_aug", [VF + 1, NVLp], dt.bfloat16)
    c1p, c2p = p["conv1"], p["conv2"]
    din("e1_src", [128, c1p.etot // 16], dt.int16)
    din("e1_oh", [128, c1p.etot], dt.float8e4)
    din("e1_ohT", [128, c1p.etot], dt.float8e4)
    din("e2_src", [128, c2p.etot // 16], dt.int16)
    din("e2_oh", [128, c2p.etot], dt.float8e4)
    din("e2_ohT", [128, c2p.etot], dt.float8e4)
    din("e1_recip", [128, c1p.nblocks], dt.float32)
    din("e1_ind", [128, c1p.nblocks], dt.float32)
    din("e2_recip", [128, c2p.nblocks], dt.float32)
    din("e2_ind", [128, c2p.nblocks], dt.float32)
    for k, v in w.items():
        dtt = dt.bfloat16 if v.dtype == BF16 else (dt.int16 if v.dtype == np.int16 else dt.float32)
        din(k, list(v.shape), dtt)
    out_d = b.dram("out", [1, NVLh], dt.bfloat16, kind="ExternalOutput")

    lp1_loc = b.dram("lp1_loc", [NVL, EMB], dt.bfloat16)
    lp1_full = b.dram("lp1_full", [NV, EMB], dt.bfloat16, addr_space="Shared")
    rp1_loc = b.dram("rp1_loc", [NCL, EMB], dt.bfloat16)
    lp2_loc = b.dram("lp2_loc", [NCL, EMB], dt.bfloat16)
    lp2_full = b.dram("lp2_full", [NC, EMB], dt.bfloat16, addr_space="Shared")
    rp2_loc = b.dram("rp2_loc", [NVL, EMB], dt.bfloat16)

    LR = mybir.ActivationFunctionType.Lrelu
    CP = mybir.ActivationFunctionType.Copy
    SQT = mybir.ActivationFunctionType.Sqrt
    AL = mybir.AluOpType

    with tile.TileContext(nc) as tc:
        nc.gpsimd.load_library(library_config.mlp)
        with (
            tc.tile_pool(name="const", bufs=1) as cpool,
            tc.tile_pool(name="resident", bufs=1) as rpool,
            tc.tile_pool(name="work", bufs=3) as wpool,
            tc.tile_pool(name="tiny", bufs=4) as tpool,
            tc.tile_pool(name="gath", bufs=3) as gpool,
            tc.tile_pool(name="ohp", bufs=2) as ohpool,
            tc.tile_pool(name="sqp", bufs=1) as sqpool,
            tc.tile_pool(name="psA", bufs=2, space="PSUM") as psA,
            tc.tile_pool(name="psT", bufs=2, space="PSUM") as psT,
            tc.tile_pool(name="psagg", bufs=3, space="PSUM") as psagg,
            tc.tile_pool(name="psout", bufs=1, space="PSUM") as psout,
        ):
            # ---- load constants into SBUF ----
            cw = {}
            for k, v in w.items():
                dtt = dt.bfloat16 if v.dtype == BF16 else dt.float32
                t = cpool.tile(list(v.shape), dtt, tag=k)
                nc.sync.dma_start(t[:], b.d[k][:])
                cw[k] = t

            ident = cw["identity"]
            zero_col = cpool.tile([128, 1], dt.float32, tag="zero_col")
            nc.vector.memset(zero_col[:], 0.0)


            # residents
            c0T = rpool.tile([128, NCLp], dt.bfloat16, tag="c0T")
            v0T = rpool.tile([128, NVLp], dt.bfloat16, tag="v0T")
            c1T = rpool.tile([128, NCLp], dt.bfloat16, tag="c1T")
            v1T = rpool.tile([128, NVLh], dt.bfloat16, tag="v1T")
            nc.vector.memset(v1T[:], 0.0)

            def transpose_to(dst_ap, src_ap, n_p, n_f):
                """dst[:n_f, :n_p] = src[:n_p, :n_f].T via PE; dst bf16 SBUF."""
                ps = psT.tile([128, 128], dt.bfloat16, tag="psT")
                nc.tensor.transpose(ps[:n_f, :n_p], src_ap, ident[:n_p, :n_p])
                nc.scalar.copy(dst_ap, ps[:n_f, :n_p])

            # =========== stage A: embeddings (sharded rows) ===========
            def embed(pre, xT_aug_name, rows_name, nrows_p, nfeat, outT, extra):
                """Two-layer embed MLP. outT <- bf16 [128, nrows_p] transposed
                result. extra: list of (wname, biasname_or_None, dram_out,
                alsoT_or_None) projections computed from outT chunks."""
                nchunks = nrows_p // 128
                # batched LN stats for all chunks: rows only feed stats (the
                # matmuls consume the transposed augmented copy)
                xall = sqpool.tile([128, nchunks, nfeat], dt.float32, tag="sqc")
                nc.sync.dma_start(xall[:], b.d[rows_name][:])
                sx = tpool.tile([128, nchunks], dt.float32, tag="esx")
                nc.vector.reduce_sum(sx[:], xall[:], axis=mybir.AxisListType.X)
                nc.vector.tensor_tensor(xall[:], xall[:], xall[:], AL.mult)
                sxx = tpool.tile([128, nchunks], dt.float32, tag="esxx")
                nc.vector.reduce_sum(sxx[:], xall[:], axis=mybir.AxisListType.X)
                inv = 1.0 / nfeat
                mu_b = tpool.tile([128, nchunks], dt.float32, tag="emub")
                nc.vector.tensor_scalar_mul(mu_b[:], sx[:], inv)
                veps = tpool.tile([128, nchunks], dt.float32, tag="evep")
                nc.vector.tensor_scalar(veps[:], sxx[:], inv, EPS, AL.mult, AL.add)
                nmusq = tpool.tile([128, nchunks], dt.float32, tag="enmu")
                nc.vector.scalar_tensor_tensor(
                    nmusq[:], mu_b[:], -1.0, mu_b[:], AL.mult, AL.mult)
                nc.vector.tensor_tensor(veps[:], veps[:], nmusq[:], AL.add)
                rstd_b = rsqrt_newton(nc, tpool, veps[:], nchunks, "eln",
                                      iters=1)
                nrstd_b = tpool.tile([128, nchunks], dt.float32, tag="enrs")
                nc.vector.tensor_scalar_mul(nrstd_b[:], rstd_b[:], -1.0)
                for ch in range(nchunks):
                    xTa = wpool.tile([nfeat + 1, 128], dt.bfloat16, tag="xTa")
                    nc.sync.dma_start(xTa[:], b.d[xT_aug_name][:, ch * 128 : (ch + 1) * 128])
                    ps = psA.tile([128, EMB], dt.float32, tag="ps")
                    nc.tensor.matmul(ps[:], xTa[:],
                                     cw[pre + "w1aug"][:], start=True, stop=True)
                    tmid = wpool.tile([128, EMB], dt.float32, tag="embmid")
                    nc.vector.scalar_tensor_tensor(
                        tmid[:], cw[pre + "r1"][:], mu_b[:, ch : ch + 1], ps[:],
                        AL.mult, AL.subtract)
                    tmid2 = wpool.tile([128, EMB], dt.float32, tag="tmid2")
                    nc.vector.scalar_tensor_tensor(
                        tmid2[:], tmid[:], nrstd_b[:, ch : ch + 1], cw[pre + "s1"][:],
                        AL.mult, AL.add)
                    z1 = wpool.tile([128, EMB], dt.bfloat16, tag="z1")
                    nc.scalar.activation(z1[:], tmid2[:], LR, bias=zero_col[:], alpha=SLOPE)
                    z1T = wpool.tile([128, 128], dt.bfloat16, tag="z1T")
                    transpose_to(z1T[:], z1[:], 128, 128)
                    ps2 = psA.tile([128, EMB], dt.float32, tag="ps")
                    nc.tensor.matmul(ps2[:], z1T[:], cw[pre + "w2"][:], start=True, stop=True)
                    u = wpool.tile([128, EMB], dt.float32, tag="embu")
                    nc.vector.tensor_add(u[:], ps2[:], cw[pre + "b2"][:])
                    z2 = wpool.tile([128, EMB], dt.bfloat16, tag="z2")
                    nc.scalar.activation(z2[:], u[:], LR, bias=zero_col[:], alpha=SLOPE)
                    transpose_to(outT[:, ch * 128 : (ch + 1) * 128], z2[:], 128, 128)
                    # projections from outT chunk
                    for (wname, bname, dout, n_valid) in extra:
                        lo = ch * 128
                        nv = min(128, max(0, n_valid - lo))
                        if nv == 0:
                            continue
                        ps3 = psA.tile([128, EMB], dt.float32, tag="ps")
                        nc.tensor.matmul(ps3[:], outT[:, lo : lo + 128],
                                         cw[wname][:], start=True, stop=True)
                        ob = wpool.tile([128, EMB], dt.bfloat16, tag="projo")
                        if bname is not None:
                            ub = wpool.tile([128, EMB], dt.float32, tag="proju")
                            nc.vector.tensor_add(ub[:], ps3[:], cw[bname][:])
                            nc.scalar.copy(ob[:], ub[:])
                        else:
                            nc.scalar.copy(ob[:], ps3[:])
                        nc.sync.dma_start(b.d[dout][lo : lo + nv, :], ob[:nv, :])

            KSTAGE = os.environ.get("KSTAGE", "full")
            embed("ve_", "varT_aug", "var_rows", NVLp, VF, v0T,
                  [("vc_wl", "vc_bl", "lp1_loc", NVL), ("cv_wr", None, "rp2_loc", NVL)])
            # all-gather lp1 early (overlaps cons embed + conv prep)
            if KSTAGE != "A":
                nc.gpsimd.collective_compute(
                    "AllGather", AL.bypass, ins=[lp1_loc[:]], outs=[lp1_full[:]],
                    replica_groups=[list(range(NCORES))])
            embed("ce_", "consT_aug", "cons_rows", NCLp, CF, c0T,
                  [("vc_wr", None, "rp1_loc", NCL)])

            # =========== conv edge stage ===========
            def conv_edges(cv, pre, lp_dram, rp_dram, src_d, oh_d, ohT_d, acc,
                           n_valid, qsel):
                fl_triv = p[pre + "fl_trivial"]
                rp_tiles = {}

                def get_rp(blk):
                    if blk in rp_tiles:
                        return rp_tiles[blk]
                    rp_sb = wpool.tile([128, EMB], dt.bfloat16, tag="rpblk")
                    lo = blk * 128
                    nv = min(128, n_valid - lo)
                    if nv < 128:
                        nc.vector.memset(rp_sb[:], 0.0)
                    nc.sync.dma_start(rp_sb[:nv, :], rp_dram[lo : lo + nv, :])
                    rp_tiles[blk] = rp_sb
                    return rp_sb

                for stream in cv.streams:
                    rp_tiles.clear()
                    base_edge = stream["start_edge"]
                    ntiles = stream["ntiles"]
                    view_lo = HI_BASE if stream["u"] == 1 else 0
                    lp_view = lp_dram[view_lo:, :] if view_lo else lp_dram[:, :]
                    blk_of_tile = {}
                    t0 = 0
                    for (blk, tcnt) in stream["blocks"]:
                        for t in range(t0, t0 + tcnt):
                            blk_of_tile[t] = (blk, t == t0, t == t0 + tcnt - 1)
                        t0 += tcnt
                    cur_ps = [None]

                    def fetch(tdone, tcn):
                        """Issue sidx DMA + gather + oh loads for one chunk."""
                        e0 = base_edge + tdone * 128
                        ne = tcn * 128
                        sidx = gpool.tile([128, ne // 16], dt.int16, tag="sidx")
                        nc.sync.dma_start(sidx[:], src_d[:, e0 // 16 : (e0 + ne) // 16])
                        sbuf = gpool.tile([128, ne // 128, EMB], dt.bfloat16,
                                          tag="sgat")
                        nc.gpsimd.dma_gather(sbuf[:], lp_view, sidx[:], ne, ne, EMB,
                                             single_packet=False, queue_num=qsel[0])
                        qsel[0] ^= 1
                        ohe = ohpool.tile([128, ne], dt.float8e4, tag="ohe")
                        nc.sync.dma_start(ohe[:], oh_d[:, e0 : e0 + ne])
                        ohT = ohpool.tile([128, ne], dt.float8e4, tag="ohT")
                        nc.sync.dma_start(ohT[:], ohT_d[:, e0 : e0 + ne])
                        return sbuf, ohe, ohT

                    def process(t0c, tcn, sbuf, ohe, ohT):
                        xw_c = ohpool.tile([128, tcn, EMB], dt.bfloat16, tag="xwc")

                        # pass A: rp broadcast (PE) + add in 4-tile groups (DVE)
                        gi = 0
                        while gi < tcn:
                            gn = min(4, tcn - gi)
                            psg = psA.tile([128, 4, EMB], dt.float32, tag="ps")
                            for k in range(gn):
                                ti = gi + k
                                blk, _, _ = blk_of_tile[t0c + ti]
                                rp_sb = get_rp(blk)
                                nc.tensor.matmul(psg[:, k, :],
                                                 ohT[:, ti * 128 : (ti + 1) * 128],
                                                 rp_sb[:], start=True, stop=True)
                            nc.vector.tensor_tensor(
                                xw_c[:, gi : gi + gn, :], sbuf[:, gi : gi + gn, :],
                                psg[:, :gn, :], AL.add)
                            gi += gn

                        # chunk stats: square + pair-sum + 3D reduces; rsqrt on
                        # ScalarE to keep DVE chains short
                        sq = sqpool.tile([128, CHUNK_TILES, EMB], dt.bfloat16,
                                         tag="sqc")
                        nc.vector.tensor_tensor(sq[:, :tcn, :], xw_c[:], xw_c[:],
                                                AL.mult)
                        xh = sqpool.tile([128, CHUNK_TILES, EMB // 2], dt.bfloat16,
                                         tag="xhc")
                        nc.vector.tensor_tensor(
                            xh[:, :tcn, :], xw_c[:, :, : EMB // 2],
                            xw_c[:, :, EMB // 2 :], AL.add)
                        sqh = sqpool.tile([128, CHUNK_TILES, EMB // 2], dt.bfloat16,
                                          tag="pub")
                        nc.vector.tensor_tensor(
                            sqh[:, :tcn, :], sq[:, :tcn, : EMB // 2],
                            sq[:, :tcn, EMB // 2 :], AL.add)
                        sx = tpool.tile([128, tcn], dt.float32, tag="sxc")
                        nc.vector.reduce_sum(sx[:], xh[:, :tcn, :],
                                             axis=mybir.AxisListType.X)
                        sxx = tpool.tile([128, tcn], dt.float32, tag="sxxc")
                        nc.vector.reduce_sum(sxx[:], sqh[:, :tcn, :],
                                             axis=mybir.AxisListType.X)
                        inv = 1.0 / EMB
                        mu = tpool.tile([128, tcn], dt.float32, tag="muc")
                        nc.vector.tensor_scalar_mul(mu[:], sx[:], inv)
                        veps = tpool.tile([128, tcn], dt.float32, tag="vepsc")
                        nc.vector.tensor_scalar(veps[:], sxx[:], inv, EPS,
                                                AL.mult, AL.add)
                        nmusq = tpool.tile([128, tcn], dt.float32, tag="nmusqc")
                        nc.vector.scalar_tensor_tensor(
                            nmusq[:], mu[:], -1.0, mu[:], AL.mult, AL.mult)
                        nc.vector.tensor_tensor(veps[:], veps[:], nmusq[:], AL.add)
                        rstd_t = rsqrt_newton(nc, tpool, veps[:], tcn, "cvr",
                                              iters=1)
                        nmr_c = tpool.tile([128, tcn], dt.float32, tag="nmrc")
                        nc.vector.scalar_tensor_tensor(
                            nmr_c[:], mu[:], -1.0, rstd_t[:], AL.mult, AL.mult)

                        # pass B: apply (ACT) + aggregate (PE)
                        for ti in range(tcn):
                            blk, isfirst, islast = blk_of_tile[t0c + ti]
                            act = wpool.tile([128, EMB], dt.bfloat16, tag="act")
                            if fl_triv:
                                nc.scalar.activation(
                                    act[:], xw_c[:, ti, :], LR,
                                    bias=nmr_c[:, ti : ti + 1],
                                    scale=rstd_t[:, ti : ti + 1], alpha=SLOPE)
                            else:
                                y1 = wpool.tile([128, EMB], dt.float32, tag="y1")
                                nc.vector.tensor_scalar(
                                    y1[:], xw_c[:, ti, :], mu[:, ti : ti + 1],
                                    rstd_t[:, ti : ti + 1], AL.subtract, AL.mult)
                                y2 = wpool.tile([128, EMB], dt.float32, tag="y2")
                                nc.vector.scalar_tensor_tensor(
                                    y2[:], y1[:], 1.0, cw[pre + "flg"][:], AL.mult, AL.mult)
                                y3 = wpool.tile([128, EMB], dt.float32, tag="y3")
                                nc.vector.tensor_add(y3[:], y2[:], cw[pre + "flb"][:])
                                nc.scalar.activation(act[:], y3[:], LR,
                                                     bias=zero_col[:], alpha=SLOPE)
                            if cur_ps[0] is None:
                                psb_new = psagg.tile([128, EMB], dt.float32, tag="agg")
                                cur_ps[0] = psb_new
                            psb = cur_ps[0]
                            nc.tensor.matmul(
                                psb[:], ohe[:, ti * 128 : (ti + 1) * 128], act[:],
                                start=isfirst, stop=islast)
                            if islast:
                                nc.vector.tensor_add(acc[:, blk, :], acc[:, blk, :], psb[:])
                                cur_ps[0] = None

                    tdone = 0
                    while tdone < ntiles:
                        # issue gathers in q0/q1 pairs so the two SWDGE queues'
                        # Q7 core pairs generate descriptors concurrently
                        batch = []
                        for _ in range(2):
                            if tdone >= ntiles:
                                break
                            tcn = min(CHUNK_TILES, ntiles - tdone)
                            batch.append((tdone, tcn, fetch(tdone, tcn)))
                            tdone += tcn
                        for (t0c, tcn, (sbuf, ohe, ohT)) in batch:
                            process(t0c, tcn, sbuf, ohe, ohT)

            # =========== post-conv: mean -> wf -> LN -> MLP ===========
            def conv_post(cv, pre, acc, rightT, outT, lpout_name, lpout_w, lpout_b,
                          n_valid, recip_sb, ind_sb):
                nblocks = cv.nblocks
                for g0 in range(0, nblocks, 8):
                    gb = min(8, nblocks - g0)
                    ub = sqpool.tile([128, 8, EMB], dt.float32, tag="pub")
                    for k in range(gb):
                        blk = g0 + k
                        mean = wpool.tile([128, EMB], dt.bfloat16, tag="mean")
                        nc.vector.tensor_scalar_mul(mean[:], acc[:, blk, :],
                                                    recip_sb[:, blk : blk + 1])
                        meanT = wpool.tile([128, 128], dt.bfloat16, tag="meanT")
                        transpose_to(meanT[:], mean[:], 128, 128)
                        ps = psA.tile([128, EMB], dt.float32, tag="ps")
                        nc.tensor.matmul(ps[:], meanT[:], cw[pre + "wf"][:],
                                         start=True, stop=True)
                        nc.vector.scalar_tensor_tensor(
                            ub[:, k, :], cw[pre + "bf"][:], ind_sb[:, blk : blk + 1],
                            ps[:], AL.mult, AL.add)
                    # batched LN stats over the group (plg/plb folded into wo1a/bo1)
                    psx = tpool.tile([128, 8], dt.float32, tag="psx")
                    nc.vector.reduce_sum(psx[:, :gb], ub[:, :gb, :],
                                         axis=mybir.AxisListType.X)
                    sqg = sqpool.tile([128, CHUNK_TILES, EMB], dt.bfloat16, tag="sqc")
                    nc.vector.tensor_tensor(sqg[:, :gb, :], ub[:, :gb, :],
                                            ub[:, :gb, :], AL.mult)
                    psxx = tpool.tile([128, 8], dt.float32, tag="psxx")
                    nc.vector.reduce_sum(psxx[:, :gb], sqg[:, :gb, :],
                                         axis=mybir.AxisListType.X)
                    inv = 1.0 / EMB
                    pmu = tpool.tile([128, 8], dt.float32, tag="pmu")
                    nc.vector.tensor_scalar_mul(pmu[:, :gb], psx[:, :gb], inv)
                    pveps = tpool.tile([128, 8], dt.float32, tag="pveps")
                    nc.vector.tensor_scalar(pveps[:, :gb], psxx[:, :gb], inv, EPS,
                                            AL.mult, AL.add)
                    pnmusq = tpool.tile([128, 8], dt.float32, tag="pnmusq")
                    nc.vector.scalar_tensor_tensor(
                        pnmusq[:, :gb], pmu[:, :gb], -1.0, pmu[:, :gb],
                        AL.mult, AL.mult)
                    nc.vector.tensor_tensor(pveps[:, :gb], pveps[:, :gb],
                                            pnmusq[:, :gb], AL.add)
                    prstd_t = rsqrt_newton(nc, tpool, pveps[:, :gb], gb, "pln",
                                           iters=1)
                    for k in range(gb):
                        blk = g0 + k
                        lo = blk * 128
                        nv = min(128, n_valid - lo)
                        lnv = wpool.tile([128, EMB], dt.bfloat16, tag="lnv")
                        nc.vector.tensor_scalar(
                            lnv[:], ub[:, k, :], pmu[:, k : k + 1],
                            prstd_t[:, k : k + 1], AL.subtract, AL.mult)
                        lnT = wpool.tile([128, 128], dt.bfloat16, tag="lnT")
                        transpose_to(lnT[:], lnv[:], 128, 128)
                        ps2 = psA.tile([128, EMB], dt.float32, tag="ps")
                        nc.tensor.matmul(ps2[:], lnT[:], cw[pre + "wo1a"][:],
                                         start=True, stop=False)
                        nc.tensor.matmul(ps2[:], rightT[:, lo : lo + 128],
                                         cw[pre + "wo1b"][:], start=False, stop=True)
                        u2 = wpool.tile([128, EMB], dt.float32, tag="pcu2")
                        nc.vector.tensor_add(u2[:], ps2[:], cw[pre + "bo1"][:])
                        tml = wpool.tile([128, EMB], dt.bfloat16, tag="tml")
                        nc.scalar.activation(tml[:], u2[:], LR, bias=zero_col[:],
                                             alpha=SLOPE)
                        tT = wpool.tile([128, 128], dt.bfloat16, tag="tT")
                        transpose_to(tT[:], tml[:], 128, 128)
                        ps3 = psA.tile([128, EMB], dt.float32, tag="ps")
                        nc.tensor.matmul(ps3[:], tT[:], cw[pre + "wo2"][:],
                                         start=True, stop=True)
                        u3 = wpool.tile([128, EMB], dt.float32, tag="pcu3")
                        nc.vector.tensor_add(u3[:], ps3[:], cw[pre + "bo2"][:])
                        res = wpool.tile([128, EMB], dt.bfloat16, tag="res")
                        nc.scalar.copy(res[:], u3[:])
                        transpose_to(outT[:, lo : lo + 128], res[:], 128, 128)
                        if lpout_name is not None and nv > 0:
                            ps4 = psA.tile([128, EMB], dt.float32, tag="ps")
                            nc.tensor.matmul(ps4[:], outT[:, lo : lo + 128],
                                             cw[lpout_w][:], start=True, stop=True)
                            ub4 = wpool.tile([128, EMB], dt.float32, tag="pc4u")
                            nc.vector.tensor_add(ub4[:], ps4[:], cw[lpout_b][:])
                            ob = wpool.tile([128, EMB], dt.bfloat16, tag="pc4o")
                            nc.scalar.copy(ob[:], ub4[:])
                            nc.sync.dma_start(b.d[lpout_name][lo : lo + nv, :],
                                              ob[:nv, :])

            # host-known scatter-mean counts
            rec1 = cpool.tile([128, c1p.nblocks], dt.float32, tag="rec1")
            nc.sync.dma_start(rec1[:], b.d["e1_recip"][:])
            ind1 = cpool.tile([128, c1p.nblocks], dt.float32, tag="ind1")
            nc.sync.dma_start(ind1[:], b.d["e1_ind"][:])
            rec2 = cpool.tile([128, c2p.nblocks], dt.float32, tag="rec2")
            nc.sync.dma_start(rec2[:], b.d["e2_recip"][:])
            ind2 = cpool.tile([128, c2p.nblocks], dt.float32, tag="ind2")
            nc.sync.dma_start(ind2[:], b.d["e2_ind"][:])

            qsel = [0]
            # conv1
            acc1 = rpool.tile([128, c1p.nblocks, EMB], dt.float32, tag="acc1")
            nc.vector.memset(acc1[:], 0.0)
            if KSTAGE not in ("A", "AG1"):
                _lp1src = rp1_loc if KSTAGE == "C1local" else lp1_full
                conv_edges(c1p, "vc_", _lp1src, rp1_loc, b.d["e1_src"],
                           b.d["e1_oh"], b.d["e1_ohT"], acc1, NCL, qsel)
            if KSTAGE not in ("A", "AG1", "C1"):
                conv_post(c1p, "vc_", acc1, c0T, c1T, "lp2_loc", "cv_wl", "cv_bl",
                          NCL, rec1, ind1)
                nc.gpsimd.collective_compute(
                    "AllGather", AL.bypass, ins=[lp2_loc[:]], outs=[lp2_full[:]],
                    replica_groups=[list(range(NCORES))])
            # conv2
            acc2 = rpool.tile([128, c2p.nblocks, EMB], dt.float32, tag="acc2")
            nc.vector.memset(acc2[:], 0.0)
            if KSTAGE not in ("A", "AG1", "C1", "P1"):
                conv_edges(c2p, "cv_", lp2_full, rp2_loc, b.d["e2_src"],
                           b.d["e2_oh"], b.d["e2_ohT"], acc2, NVL, qsel)
                conv_post(c2p, "cv_", acc2, v0T, v1T, None, None, None,
                          NVL, rec2, ind2)

            # =========== heads ===========
            if KSTAGE != "full" or nact == 0:
                zrow = wpool.tile([1, 512], dt.bfloat16, tag="orow")
                nc.vector.memset(zrow[:], 0.0)
                for j in range(NVLh // 512):
                    nc.sync.dma_start(out_d[:, j * 512 : (j + 1) * 512], zrow[:])
            else:
                nch = NVLh // 512
                for j in range(nch):
                    pso = psout.tile([1, 512], dt.float32, tag="pso")
                    for hi in range(nact):
                        ps = psA.tile([128, 512], dt.float32, tag="ps")
                        nc.tensor.matmul(ps[:], cw["hw1"][:, hi, :],
                                         v1T[:, j * 512 : (j + 1) * 512],
                                         start=True, stop=True)
                        hh = wpool.tile([128, 512], dt.bfloat16, tag="hh")
                        nc.scalar.activation(hh[:], ps[:], LR,
                                             bias=cw["hb1"][:, hi : hi + 1],
                                             scale=1.0, alpha=SLOPE)
                        nc.tensor.matmul(pso[:], cw["hw2"][:, hi : hi + 1], hh[:],
                                         start=(hi == 0), stop=(hi == nact - 1))
                    # raw head sum; host applies out_scale/out_add affine
                    orow = cpool.tile([1, 512], dt.bfloat16, tag="orow")
                    nc.scalar.copy(orow[:], pso[:])
                    nc.sync.dma_start(out_d[:, j * 512 : (j + 1) * 512], orow[:])

    nc.compile()
    return b


_CACHE = {}


def kernel(**inputs):
    key = tuple(sorted((k, tuple(np.asarray(v).shape)) for k, v in inputs.items()))
    p = host_prep(inputs)
    ck = (key, p["nact"], p["conv1"].etot, p["conv2"].etot,
          p["vc_fl_trivial"], p["cv_fl_trivial"])
    if ck in _CACHE:
        b = _CACHE[ck]
    else:
        b = build_program(p)
        _CACHE[ck] = b
    in_maps = [dict(p["core_inputs"][c]) for c in range(NCORES)]
    res = run_bass_kernel_spmd(b.nc, in_maps, core_ids=list(range(NCORES)))
    NVL = p["NVL"]
    out = np.concatenate([res.results[c]["out"][0, :NVL] for c in range(NCORES)])
    out = out.astype(np.float32) * p["out_scale"] + p["out_add"]
    return out.astype(np.float32)



# revision 37
# speedup vs baseline: 1.2111x; 1.2111x over previous
"""Trainium2 Bass kernel for BipartiteGCN (8 NeuronCores, SPMD).

Strategy:
 - Node rows sharded 8 ways (cons: NC/8 rows per core, var: NV/8).
 - Edges sharded by DESTINATION range; per-core edges sorted by dst block
   (128 dst rows per block), bucketed by src>=32768 where needed (int16
   gather indices).
 - Per-edge pipeline: dma_gather of lp[src] and rp[dst] rows (bf16),
   add -> LayerNorm (bn_stats) -> LeakyReLU fused on ScalarE -> one-hot
   (iota is_equal) -> matmul-accumulate into PSUM per dst block: computes
   segment-sum AND counts (ones column) with no scatter DMA.
 - Linearity: wf/bias applied after the segment-mean (per node, not per
   edge).
 - Only lp tables are all-gathered; rp/skip/post-MLP/head stages stay
   local to each core's dst shard. Output is the var shard -> host concat.
"""

import os
import sys

for _p in ("/opt/trn_rl_repo",):
    if _p not in sys.path:
        sys.path.insert(0, _p)

import numpy as np
import ml_dtypes

import concourse.bass as bass
import concourse.bacc as bacc
import concourse.mybir as mybir
from concourse import tile, library_config
from concourse.bass_utils import run_bass_kernel_spmd
from concourse import hw_specs as _hw_specs

# The stock 0.34 ns/descriptor SWDGE estimate is calibrated on plain SWDGE
# dma_start; the Q7 dma_gather firmware measures ~7.7 ns/descriptor. With the
# low estimate the Tile scheduler orders consumers of a just-issued gather
# ahead of ready work, causing head-of-line blocking in engine FIFOs.
_hw_specs.TRN2Spec.SWDGE_NS_PER_DESCRIPTOR = 7.7

BF16 = ml_dtypes.bfloat16
F32 = np.float32
NCORES = 8
EMB = 128
CHUNK_TILES = 32     # tiles (128 edges) per dma_gather call (4096 edges)
HI_BASE = 32768
EPS = 1e-5
SLOPE = 0.01

dt = mybir.dt


def _wrap_idx(idx_i16):
    """[N] int16 -> [128, N//16] wrapped (i at [i%16, i//16]) + replicated 8x."""
    n = idx_i16.shape[0]
    assert n % 16 == 0
    w = idx_i16.reshape(n // 16, 16).T
    return np.tile(w, (8, 1)).copy()


def _lane_major(arr, lanes=128):
    """[N] -> [lanes, N//lanes] with element i at [i%lanes, i//lanes]."""
    n = arr.shape[0]
    assert n % lanes == 0
    return arr.reshape(n // lanes, lanes).T.copy()


def _bcast_row(v, rows=128):
    """[F] -> [rows, F] replicated, f32."""
    return np.broadcast_to(np.asarray(v, F32)[None, :], (rows, v.shape[0])).copy()


class ConvPrep:
    """Per-conv edge-sharding data. Same segment layout for all cores."""

    def __init__(self, dst, src, n_dst, n_src, dst_per_core):
        self.n_dst_local = dst_per_core
        self.nblocks = -(-dst_per_core // 128)
        self.two_buckets = n_src > HI_BASE
        nb = self.nblocks
        nu = 2 if self.two_buckets else 1

        core = dst // dst_per_core
        dloc_all = dst - core * dst_per_core
        block_all = dloc_all // 128

        # per (core, bucket, block) edge lists
        per = [[[None] * nb for _ in range(nu)] for _ in range(NCORES)]
        for c in range(NCORES):
            m = core == c
            d_c = dloc_all[m]
            s_c = src[m]
            b_c = block_all[m]
            u_c = (s_c >= HI_BASE).astype(np.int8) if self.two_buckets else np.zeros(
                len(s_c), np.int8
            )
            for u in range(nu):
                mu = u_c == u
                db, sb, bb = d_c[mu], s_c[mu], b_c[mu]
                order = np.argsort(bb, kind="stable")
                db, sb, bb = db[order], sb[order], bb[order]
                bounds = np.searchsorted(bb, np.arange(nb + 1))
                for b in range(nb):
                    lo, hi = bounds[b], bounds[b + 1]
                    per[c][u][b] = (sb[lo:hi], db[lo:hi])

        # uniform tile counts (128 edges): max over cores
        self.ntiles = np.zeros((nu, nb), np.int64)
        for u in range(nu):
            for b in range(nb):
                mx = max(len(per[c][u][b][0]) for c in range(NCORES))
                self.ntiles[u, b] = -(-mx // 128) if mx > 0 else 0

        etot = int(self.ntiles.sum()) * 128
        self.etot = etot

        # host-known scatter-mean counts per core: [128, nb] lane-major
        self.recip = np.zeros((NCORES, 128, nb), F32)
        self.ind = np.zeros((NCORES, 128, nb), F32)
        for c in range(NCORES):
            cnt = np.bincount(dloc_all[core == c], minlength=nb * 128).astype(F32)
            cnt = cnt[: nb * 128].reshape(nb, 128).T  # [128, nb]
            self.recip[c] = 1.0 / np.maximum(cnt, 1.0)
            self.ind[c] = np.minimum(cnt, 1.0)

        # build padded per-core arrays in segment order (u-major, b-minor)
        self.src_idx = np.zeros((NCORES, etot), np.int16)
        self.dstrel = np.full((NCORES, etot), -1.0, F32)
        off = 0
        self.seg_offsets = {}
        for u in range(nu):
            for b in range(nb):
                g = int(self.ntiles[u, b])
                if g == 0:
                    continue
                self.seg_offsets[(u, b)] = off
                for c in range(NCORES):
                    sb, db = per[c][u][b]
                    n = len(sb)
                    s_adj = sb - (HI_BASE if u == 1 else 0)
                    self.src_idx[c, off : off + n] = s_adj.astype(np.int16)
                    self.dstrel[c, off : off + n] = (db - 128 * b).astype(F32)
                off += g * 128
        assert off == etot

        # stream layout: per bucket, list of (block, ntiles)
        self.streams = []
        for u in range(nu):
            blocks = [(b, int(self.ntiles[u, b])) for b in range(nb) if self.ntiles[u, b] > 0]
            start = self.seg_offsets[(u, blocks[0][0])] if blocks else 0
            nt = sum(g for _, g in blocks)
            self.streams.append({"u": u, "blocks": blocks, "start_edge": start, "ntiles": nt})

    def core_arrays(self, c):
        dr = self.dstrel[c]
        i = np.nonzero(dr >= 0)[0]
        lane = i % 128
        tb = (i // 128) * 128
        d = dr[i].astype(np.int64)
        oh = np.zeros((128, self.etot), ml_dtypes.float8_e4m3)
        oh[lane, tb + d] = 1.0
        ohT = np.zeros((128, self.etot), ml_dtypes.float8_e4m3)
        ohT[d, tb + lane] = 1.0
        return _wrap_idx(self.src_idx[c]), oh, ohT


def host_prep(inputs):
    p = {}
    cons_x = np.asarray(inputs["cons_x"], F32)
    var_x = np.asarray(inputs["var_x"], F32)
    edge_cons = np.asarray(inputs["edge_cons"]).astype(np.int64)
    edge_var = np.asarray(inputs["edge_var"]).astype(np.int64)
    head_mask = np.asarray(inputs["head_mask"]).astype(bool)

    NC, CF = cons_x.shape
    NV, VF = var_x.shape
    assert NC % NCORES == 0 and NV % NCORES == 0
    NCL, NVL = NC // NCORES, NV // NCORES
    p.update(NC=NC, NV=NV, CF=CF, VF=VF, NCL=NCL, NVL=NVL)

    # conv1: v->c (src=edge_var over NV, dst=edge_cons over NC)
    p["conv1"] = ConvPrep(edge_cons, edge_var, NC, NV, NCL)
    # conv2: c->v
    p["conv2"] = ConvPrep(edge_var, edge_cons, NV, NC, NVL)

    # ---- weights ----
    w = {}

    def embed_w(prefix, g, b, w1, b1, w2, b2, feat):
        w1 = np.asarray(w1, F32)
        w1g = np.asarray(g, F32)[:, None] * w1
        aug = np.concatenate([w1g, np.zeros((1, w1.shape[1]), F32)], 0)
        w[prefix + "w1aug"] = aug.astype(BF16)
        w[prefix + "s1"] = _bcast_row(np.asarray(b, F32) @ w1 + np.asarray(b1, F32))
        w[prefix + "r1"] = _bcast_row(w1g.sum(0))
        w[prefix + "w2"] = np.asarray(w2, F32).astype(BF16)
        w[prefix + "b2"] = _bcast_row(np.asarray(b2, F32))

    embed_w("ce_", inputs["ce_ln_g"], inputs["ce_ln_b"], inputs["ce_w1"],
            inputs["ce_b1"], inputs["ce_w2"], inputs["ce_b2"], CF)
    embed_w("ve_", inputs["ve_ln_g"], inputs["ve_ln_b"], inputs["ve_w1"],
            inputs["ve_b1"], inputs["ve_w2"], inputs["ve_b2"], VF)

    for pre in ("vc_", "cv_"):
        wl = np.asarray(inputs[pre + "wl"], F32)
        w[pre + "wl"] = wl.astype(BF16)
        w[pre + "bl"] = _bcast_row(np.asarray(inputs[pre + "bl"], F32))
        w[pre + "wr"] = np.asarray(inputs[pre + "wr"], F32).astype(BF16)
        flg = np.asarray(inputs[pre + "flg"], F32)
        flb = np.asarray(inputs[pre + "flb"], F32)
        p[pre + "fl_trivial"] = bool(np.all(flg == 1.0) and np.all(flb == 0.0))
        w[pre + "flg"] = _bcast_row(flg)
        w[pre + "flb"] = _bcast_row(flb)
        w[pre + "wf"] = np.asarray(inputs[pre + "wf"], F32).astype(BF16)
        w[pre + "bf"] = _bcast_row(np.asarray(inputs[pre + "bf"], F32))
        wo1 = np.asarray(inputs[pre + "wo1"], F32)
        plg = np.asarray(inputs[pre + "plg"], F32)
        plb = np.asarray(inputs[pre + "plb"], F32)
        w[pre + "wo1a"] = (plg[:, None] * wo1[:EMB]).astype(BF16)
        w[pre + "wo1b"] = wo1[EMB:].astype(BF16)
        w[pre + "bo1"] = _bcast_row(np.asarray(inputs[pre + "bo1"], F32) + plb @ wo1[:EMB])
        w[pre + "wo2"] = np.asarray(inputs[pre + "wo2"], F32).astype(BF16)
        w[pre + "bo2"] = _bcast_row(np.asarray(inputs[pre + "bo2"], F32))

    # heads
    active = np.nonzero(head_mask)[0]
    nact = int(len(active))
    p["nact"] = nact
    denom = max(float(head_mask.sum()), 1.0)
    hb2 = np.asarray(inputs["hb2"], F32)
    p["out_scale"] = 1.0 / denom
    p["out_add"] = float(hb2[active].sum() / denom)
    if nact > 0:
        hw1 = np.asarray(inputs["hw1"], F32)[active]          # [nact,128,128]
        w["hw1"] = hw1.transpose(1, 0, 2).astype(BF16).copy()  # [128,nact,128]
        w["hb1"] = np.asarray(inputs["hb1"], F32)[active].T.copy()   # [128,nact]
        w["hw2"] = np.asarray(inputs["hw2"], F32)[active].T.astype(BF16).copy()  # [128,nact]

    w["identity"] = np.eye(128, dtype=BF16)
    p["weights"] = w

    # ---- per-core inputs ----
    NCLp = -(-NCL // 128) * 128
    NVLp = -(-NVL // 128) * 128
    p.update(NCLp=NCLp, NVLp=NVLp)
    NVLh = -(-NVL // 512) * 512  # head stage col padding
    p["NVLh"] = NVLh

    core_inputs = []
    for c in range(NCORES):
        m = {}
        cx = cons_x[c * NCL : (c + 1) * NCL]
        vx = var_x[c * NVL : (c + 1) * NVL]
        cxp = np.zeros((NCLp, CF), F32)
        cxp[:NCL] = cx
        vxp = np.zeros((NVLp, VF), F32)
        vxp[:NVL] = vx
        m["cons_rows"] = cxp.reshape(NCLp // 128, 128, CF).transpose(1, 0, 2).copy()
        m["var_rows"] = vxp.reshape(NVLp // 128, 128, VF).transpose(1, 0, 2).copy()
        m["consT_aug"] = np.concatenate([cxp.T, np.ones((1, NCLp), F32)], 0).astype(BF16)
        m["varT_aug"] = np.concatenate([vxp.T, np.ones((1, NVLp), F32)], 0).astype(BF16)
        s1, oh1, ohT1 = p["conv1"].core_arrays(c)
        m["e1_src"], m["e1_oh"], m["e1_ohT"] = s1, oh1, ohT1
        s2, oh2, ohT2 = p["conv2"].core_arrays(c)
        m["e2_src"], m["e2_oh"], m["e2_ohT"] = s2, oh2, ohT2
        m["e1_recip"] = p["conv1"].recip[c]
        m["e1_ind"] = p["conv1"].ind[c]
        m["e2_recip"] = p["conv2"].recip[c]
        m["e2_ind"] = p["conv2"].ind[c]
        for k, v in w.items():
            m[k] = v
        core_inputs.append(m)
    p["core_inputs"] = core_inputs
    return p


# ---------------------------------------------------------------------------
# program builder
# ---------------------------------------------------------------------------


class B:
    """Builder context."""

    def __init__(self, p):
        self.p = p
        self.nc = bacc.Bacc("TRN2", target_bir_lowering=False, debug=False,
                            num_devices=NCORES, num_swdge_queues=2)
        self.d = {}  # dram tensors

    def dram(self, name, shape, dtype, kind=None, addr_space=None):
        kw = {}
        if kind:
            kw["kind"] = kind
        if addr_space:
            kw["addr_space"] = addr_space
        t = self.nc.dram_tensor(name, list(shape), dtype, **kw)
        self.d[name] = t
        return t


MAGIC = 0x5F3759DF


def rsqrt_newton(nc, pool, src_ap, n, tag, iters=2):
    """1/sqrt(src) on DVE only. src_ap [128, n] f32 > 0."""
    AL = mybir.AluOpType
    sh = pool.tile([128, n], dt.int32, tag=tag + "sh")
    nc.vector.tensor_scalar(sh[:], src_ap.bitcast(dt.int32), 1, None,
                            AL.arith_shift_right)
    y0 = pool.tile([128, n], dt.int32, tag=tag + "y0")
    nc.vector.tensor_scalar(y0[:], sh[:], -1, MAGIC, AL.mult, AL.add)
    cur = y0[:].bitcast(dt.float32)
    h = pool.tile([128, n], dt.float32, tag=tag + "h")
    nc.vector.tensor_scalar_mul(h[:], src_ap, 0.5)
    yy = pool.tile([128, n], dt.float32, tag=tag + "yy")
    for it in range(iters):
        nc.vector.tensor_tensor(yy[:], cur, cur, AL.mult)
        nc.vector.tensor_tensor(yy[:], yy[:], h[:], AL.mult)
        nc.vector.tensor_scalar(yy[:], yy[:], -1.0, 1.5, AL.mult, AL.add)
        nxt = pool.tile([128, n], dt.float32, tag=tag + f"n{it}")
        nc.vector.tensor_tensor(nxt[:], cur, yy[:], AL.mult)
        cur = nxt[:]
    return cur


def ln_rows_stats(nc, pool, x_ap, nrows, nfeat):
    """LN stats for rows-major f32 [nrows, nfeat] -> (rstd_ap, nmr, mu). DVE only."""
    s1 = pool.tile([128, 1], dt.float32, tag="s1")
    nc.vector.reduce_sum(s1[:nrows], x_ap, axis=mybir.AxisListType.X)
    sq = pool.tile([128, nfeat], dt.float32, tag="sqscratch")
    s2 = pool.tile([128, 1], dt.float32, tag="s2")
    nc.vector.scalar_tensor_tensor(
        sq[:nrows], x_ap, 0.0, x_ap, mybir.AluOpType.add, mybir.AluOpType.mult,
        accum_out=s2[:nrows],
    )
    inv = 1.0 / nfeat
    musq = pool.tile([128, 1], dt.float32, tag="musq")
    nc.vector.scalar_tensor_tensor(
        musq[:nrows], s1[:nrows], inv * inv, s1[:nrows],
        mybir.AluOpType.mult, mybir.AluOpType.mult,
    )
    veps0 = pool.tile([128, 1], dt.float32, tag="veps0")
    nc.vector.tensor_scalar(veps0[:nrows], s2[:nrows], inv, EPS,
                            mybir.AluOpType.mult, mybir.AluOpType.add)
    veps = pool.tile([128, 1], dt.float32, tag="veps")
    nc.vector.tensor_tensor(veps[:nrows], veps0[:nrows], musq[:nrows],
                            mybir.AluOpType.subtract)
    rstd = rsqrt_newton(nc, pool, veps[:nrows], 1, "lnr")
    mu = pool.tile([128, 1], dt.float32, tag="mu")
    nc.vector.tensor_scalar_mul(mu[:nrows], s1[:nrows], inv)
    nmr = pool.tile([128, 1], dt.float32, tag="nmr")
    nc.vector.scalar_tensor_tensor(
        nmr[:nrows], mu[:nrows], -1.0, rstd,
        mybir.AluOpType.mult, mybir.AluOpType.mult,
    )
    return rstd, nmr, mu


def build_program(p):
    b = B(p)
    nc = b.nc
    w = p["weights"]
    NCL, NVL, NCLp, NVLp = p["NCL"], p["NVL"], p["NCLp"], p["NVLp"]
    CF, VF = p["CF"], p["VF"]
    NC, NV = p["NC"], p["NV"]
    NVLh = p["NVLh"]
    nact = p["nact"]

    # ---- dram declarations ----
    din = lambda n, s, t: b.dram(n, s, t, kind="ExternalInput")
    din("cons_rows", [128, NCLp // 128, CF], dt.float32)
    din("var_rows", [128, NVLp // 128, VF], dt.float32)
    din("consT_aug", [CF + 1, NCLp], dt.bfloat16)
    din("varT_aug", [VF + 1, NVLp], dt.bfloat16)
    c1p, c2p = p["conv1"], p["conv2"]
    din("e1_src", [128, c1p.etot // 16], dt.int16)
    din("e1_oh", [128, c1p.etot], dt.float8e4)
    din("e1_ohT", [128, c1p.etot], dt.float8e4)
    din("e2_src", [128, c2p.etot // 16], dt.int16)
    din("e2_oh", [128, c2p.etot], dt.float8e4)
    din("e2_ohT", [128, c2p.etot], dt.float8e4)
    din("e1_recip", [128, c1p.nblocks], dt.float32)
    din("e1_ind", [128, c1p.nblocks], dt.float32)
    din("e2_recip", [128, c2p.nblocks], dt.float32)
    din("e2_ind", [128, c2p.nblocks], dt.float32)
    for k, v in w.items():
        dtt = dt.bfloat16 if v.dtype == BF16 else (dt.int16 if v.dtype == np.int16 else dt.float32)
        din(k, list(v.shape), dtt)
    out_d = b.dram("out", [1, NVLh], dt.bfloat16, kind="ExternalOutput")

    lp1_loc = b.dram("lp1_loc", [NVL, EMB], dt.bfloat16)
    lp1_full = b.dram("lp1_full", [NV, EMB], dt.bfloat16, addr_space="Shared")
    rp1_loc = b.dram("rp1_loc", [NCL, EMB], dt.bfloat16)
    lp2_loc = b.dram("lp2_loc", [NCL, EMB], dt.bfloat16)
    lp2_full = b.dram("lp2_full", [NC, EMB], dt.bfloat16, addr_space="Shared")
    rp2_loc = b.dram("rp2_loc", [NVL, EMB], dt.bfloat16)

    LR = mybir.ActivationFunctionType.Lrelu
    CP = mybir.ActivationFunctionType.Copy
    SQT = mybir.ActivationFunctionType.Sqrt
    AL = mybir.AluOpType

    with tile.TileContext(nc) as tc:
        nc.gpsimd.load_library(library_config.mlp)
        with (
            tc.tile_pool(name="const", bufs=1) as cpool,
            tc.tile_pool(name="resident", bufs=1) as rpool,
            tc.tile_pool(name="work", bufs=3) as wpool,
            tc.tile_pool(name="tiny", bufs=4) as tpool,
            tc.tile_pool(name="gath", bufs=3) as gpool,
            tc.tile_pool(name="ohp", bufs=2) as ohpool,
            tc.tile_pool(name="sqp", bufs=1) as sqpool,
            tc.tile_pool(name="psA", bufs=2, space="PSUM") as psA,
            tc.tile_pool(name="psT", bufs=2, space="PSUM") as psT,
            tc.tile_pool(name="psagg", bufs=3, space="PSUM") as psagg,
            tc.tile_pool(name="psout", bufs=1, space="PSUM") as psout,
        ):
            # ---- load constants into SBUF ----
            cw = {}
            for k, v in w.items():
                dtt = dt.bfloat16 if v.dtype == BF16 else dt.float32
                t = cpool.tile(list(v.shape), dtt, tag=k)
                nc.sync.dma_start(t[:], b.d[k][:])
                cw[k] = t

            ident = cw["identity"]
            zero_col = cpool.tile([128, 1], dt.float32, tag="zero_col")
            nc.vector.memset(zero_col[:], 0.0)


            # residents
            c0T = rpool.tile([128, NCLp], dt.bfloat16, tag="c0T")
            v0T = rpool.tile([128, NVLp], dt.bfloat16, tag="v0T")
            c1T = rpool.tile([128, NCLp], dt.bfloat16, tag="c1T")
            v1T = rpool.tile([128, NVLh], dt.bfloat16, tag="v1T")
            nc.vector.memset(v1T[:], 0.0)

            def transpose_to(dst_ap, src_ap, n_p, n_f):
                """dst[:n_f, :n_p] = src[:n_p, :n_f].T via PE; dst bf16 SBUF."""
                ps = psT.tile([128, 128], dt.bfloat16, tag="psT")
                nc.tensor.transpose(ps[:n_f, :n_p], src_ap, ident[:n_p, :n_p])
                nc.scalar.copy(dst_ap, ps[:n_f, :n_p])

            # =========== stage A: embeddings (sharded rows) ===========
            def embed(pre, xT_aug_name, rows_name, nrows_p, nfeat, outT, extra):
                """Two-layer embed MLP. outT <- bf16 [128, nrows_p] transposed
                result. extra: list of (wname, biasname_or_None, dram_out,
                alsoT_or_None) projections computed from outT chunks."""
                nchunks = nrows_p // 128
                # batched LN stats for all chunks: rows only feed stats (the
                # matmuls consume the transposed augmented copy)
                xall = sqpool.tile([128, nchunks, nfeat], dt.float32, tag="sqc")
                nc.sync.dma_start(xall[:], b.d[rows_name][:])
                sx = tpool.tile([128, nchunks], dt.float32, tag="esx")
                nc.vector.reduce_sum(sx[:], xall[:], axis=mybir.AxisListType.X)
                nc.vector.tensor_tensor(xall[:], xall[:], xall[:], AL.mult)
                sxx = tpool.tile([128, nchunks], dt.float32, tag="esxx")
                nc.vector.reduce_sum(sxx[:], xall[:], axis=mybir.AxisListType.X)
                inv = 1.0 / nfeat
                mu_b = tpool.tile([128, nchunks], dt.float32, tag="emub")
                nc.vector.tensor_scalar_mul(mu_b[:], sx[:], inv)
                veps = tpool.tile([128, nchunks], dt.float32, tag="evep")
                nc.vector.tensor_scalar(veps[:], sxx[:], inv, EPS, AL.mult, AL.add)
                nmusq = tpool.tile([128, nchunks], dt.float32, tag="enmu")
                nc.vector.scalar_tensor_tensor(
                    nmusq[:], mu_b[:], -1.0, mu_b[:], AL.mult, AL.mult)
                nc.vector.tensor_tensor(veps[:], veps[:], nmusq[:], AL.add)
                rv_b = tpool.tile([128, nchunks], dt.float32, tag="erv")
                nc.vector.reciprocal(rv_b[:], veps[:])
                rstd_b = tpool.tile([128, nchunks], dt.float32, tag="erst")
                nc.scalar.activation(rstd_b[:], rv_b[:], SQT, bias=zero_col[:])
                nrstd_b = tpool.tile([128, nchunks], dt.float32, tag="enrs")
                nc.vector.tensor_scalar_mul(nrstd_b[:], rstd_b[:], -1.0)
                for ch in range(nchunks):
                    xTa = wpool.tile([nfeat + 1, 128], dt.bfloat16, tag="xTa")
                    nc.sync.dma_start(xTa[:], b.d[xT_aug_name][:, ch * 128 : (ch + 1) * 128])
                    ps = psA.tile([128, EMB], dt.float32, tag="ps")
                    nc.tensor.matmul(ps[:], xTa[:],
                                     cw[pre + "w1aug"][:], start=True, stop=True)
                    tmid = wpool.tile([128, EMB], dt.float32, tag="embmid")
                    nc.vector.scalar_tensor_tensor(
                        tmid[:], cw[pre + "r1"][:], mu_b[:, ch : ch + 1], ps[:],
                        AL.mult, AL.subtract)
                    tmid2 = wpool.tile([128, EMB], dt.float32, tag="tmid2")
                    nc.vector.scalar_tensor_tensor(
                        tmid2[:], tmid[:], nrstd_b[:, ch : ch + 1], cw[pre + "s1"][:],
                        AL.mult, AL.add)
                    z1 = wpool.tile([128, EMB], dt.bfloat16, tag="z1")
                    nc.scalar.activation(z1[:], tmid2[:], LR, bias=zero_col[:], alpha=SLOPE)
                    z1T = wpool.tile([128, 128], dt.bfloat16, tag="z1T")
                    transpose_to(z1T[:], z1[:], 128, 128)
                    ps2 = psA.tile([128, EMB], dt.float32, tag="ps")
                    nc.tensor.matmul(ps2[:], z1T[:], cw[pre + "w2"][:], start=True, stop=True)
                    u = wpool.tile([128, EMB], dt.float32, tag="embu")
                    nc.vector.tensor_add(u[:], ps2[:], cw[pre + "b2"][:])
                    z2 = wpool.tile([128, EMB], dt.bfloat16, tag="z2")
                    nc.scalar.activation(z2[:], u[:], LR, bias=zero_col[:], alpha=SLOPE)
                    transpose_to(outT[:, ch * 128 : (ch + 1) * 128], z2[:], 128, 128)
                    # projections from outT chunk
                    for (wname, bname, dout, n_valid) in extra:
                        lo = ch * 128
                        nv = min(128, max(0, n_valid - lo))
                        if nv == 0:
                            continue
                        ps3 = psA.tile([128, EMB], dt.float32, tag="ps")
                        nc.tensor.matmul(ps3[:], outT[:, lo : lo + 128],
                                         cw[wname][:], start=True, stop=True)
                        ob = wpool.tile([128, EMB], dt.bfloat16, tag="projo")
                        if bname is not None:
                            ub = wpool.tile([128, EMB], dt.float32, tag="proju")
                            nc.vector.tensor_add(ub[:], ps3[:], cw[bname][:])
                            nc.scalar.copy(ob[:], ub[:])
                        else:
                            nc.scalar.copy(ob[:], ps3[:])
                        nc.sync.dma_start(b.d[dout][lo : lo + nv, :], ob[:nv, :])

            KSTAGE = os.environ.get("KSTAGE", "full")
            embed("ve_", "varT_aug", "var_rows", NVLp, VF, v0T,
                  [("vc_wl", "vc_bl", "lp1_loc", NVL), ("cv_wr", None, "rp2_loc", NVL)])
            # all-gather lp1 early (overlaps cons embed + conv prep)
            if KSTAGE != "A":
                nc.gpsimd.collective_compute(
                    "AllGather", AL.bypass, ins=[lp1_loc[:]], outs=[lp1_full[:]],
                    replica_groups=[list(range(NCORES))])
            embed("ce_", "consT_aug", "cons_rows", NCLp, CF, c0T,
                  [("vc_wr", None, "rp1_loc", NCL)])

            # =========== conv edge stage ===========
            def conv_edges(cv, pre, lp_dram, rp_dram, src_d, oh_d, ohT_d, acc,
                           n_valid, qsel):
                fl_triv = p[pre + "fl_trivial"]
                rp_tiles = {}

                def get_rp(blk):
                    if blk in rp_tiles:
                        return rp_tiles[blk]
                    rp_sb = wpool.tile([128, EMB], dt.bfloat16, tag="rpblk")
                    lo = blk * 128
                    nv = min(128, n_valid - lo)
                    if nv < 128:
                        nc.vector.memset(rp_sb[:], 0.0)
                    nc.sync.dma_start(rp_sb[:nv, :], rp_dram[lo : lo + nv, :])
                    rp_tiles[blk] = rp_sb
                    return rp_sb

                for stream in cv.streams:
                    rp_tiles.clear()
                    base_edge = stream["start_edge"]
                    ntiles = stream["ntiles"]
                    view_lo = HI_BASE if stream["u"] == 1 else 0
                    lp_view = lp_dram[view_lo:, :] if view_lo else lp_dram[:, :]
                    blk_of_tile = {}
                    t0 = 0
                    for (blk, tcnt) in stream["blocks"]:
                        for t in range(t0, t0 + tcnt):
                            blk_of_tile[t] = (blk, t == t0, t == t0 + tcnt - 1)
                        t0 += tcnt
                    cur_ps = [None]

                    def fetch(tdone, tcn):
                        """Issue sidx DMA + gather + oh loads for one chunk."""
                        e0 = base_edge + tdone * 128
                        ne = tcn * 128
                        sidx = gpool.tile([128, ne // 16], dt.int16, tag="sidx")
                        nc.sync.dma_start(sidx[:], src_d[:, e0 // 16 : (e0 + ne) // 16])
                        sbuf = gpool.tile([128, ne // 128, EMB], dt.bfloat16,
                                          tag="sgat")
                        nc.gpsimd.dma_gather(sbuf[:], lp_view, sidx[:], ne, ne, EMB,
                                             single_packet=False, queue_num=qsel[0])
                        qsel[0] ^= 1
                        ohe = ohpool.tile([128, ne], dt.float8e4, tag="ohe")
                        nc.sync.dma_start(ohe[:], oh_d[:, e0 : e0 + ne])
                        ohT = ohpool.tile([128, ne], dt.float8e4, tag="ohT")
                        nc.sync.dma_start(ohT[:], ohT_d[:, e0 : e0 + ne])
                        return sbuf, ohe, ohT

                    def process(t0c, tcn, sbuf, ohe, ohT):
                        xw_c = ohpool.tile([128, tcn, EMB], dt.bfloat16, tag="xwc")

                        # pass A: rp broadcast (PE) + add in 4-tile groups (DVE)
                        gi = 0
                        while gi < tcn:
                            gn = min(4, tcn - gi)
                            psg = psA.tile([128, 4, EMB], dt.float32, tag="ps")
                            for k in range(gn):
                                ti = gi + k
                                blk, _, _ = blk_of_tile[t0c + ti]
                                rp_sb = get_rp(blk)
                                nc.tensor.matmul(psg[:, k, :],
                                                 ohT[:, ti * 128 : (ti + 1) * 128],
                                                 rp_sb[:], start=True, stop=True)
                            nc.vector.tensor_tensor(
                                xw_c[:, gi : gi + gn, :], sbuf[:, gi : gi + gn, :],
                                psg[:, :gn, :], AL.add)
                            gi += gn

                        # chunk stats: square + pair-sum + 3D reduces; rsqrt on
                        # ScalarE to keep DVE chains short
                        sq = sqpool.tile([128, CHUNK_TILES, EMB], dt.bfloat16,
                                         tag="sqc")
                        nc.vector.tensor_tensor(sq[:, :tcn, :], xw_c[:], xw_c[:],
                                                AL.mult)
                        xh = sqpool.tile([128, CHUNK_TILES, EMB // 2], dt.bfloat16,
                                         tag="xhc")
                        nc.vector.tensor_tensor(
                            xh[:, :tcn, :], xw_c[:, :, : EMB // 2],
                            xw_c[:, :, EMB // 2 :], AL.add)
                        sqh = sqpool.tile([128, CHUNK_TILES, EMB // 2], dt.bfloat16,
                                          tag="pub")
                        nc.vector.tensor_tensor(
                            sqh[:, :tcn, :], sq[:, :tcn, : EMB // 2],
                            sq[:, :tcn, EMB // 2 :], AL.add)
                        sx = tpool.tile([128, tcn], dt.float32, tag="sxc")
                        nc.vector.reduce_sum(sx[:], xh[:, :tcn, :],
                                             axis=mybir.AxisListType.X)
                        sxx = tpool.tile([128, tcn], dt.float32, tag="sxxc")
                        nc.vector.reduce_sum(sxx[:], sqh[:, :tcn, :],
                                             axis=mybir.AxisListType.X)
                        inv = 1.0 / EMB
                        mu = tpool.tile([128, tcn], dt.float32, tag="muc")
                        nc.vector.tensor_scalar_mul(mu[:], sx[:], inv)
                        veps = tpool.tile([128, tcn], dt.float32, tag="vepsc")
                        nc.vector.tensor_scalar(veps[:], sxx[:], inv, EPS,
                                                AL.mult, AL.add)
                        nmusq = tpool.tile([128, tcn], dt.float32, tag="nmusqc")
                        nc.vector.scalar_tensor_tensor(
                            nmusq[:], mu[:], -1.0, mu[:], AL.mult, AL.mult)
                        nc.vector.tensor_tensor(veps[:], veps[:], nmusq[:], AL.add)
                        rveps = tpool.tile([128, tcn], dt.float32, tag="rvepsc")
                        nc.vector.reciprocal(rveps[:], veps[:])
                        rstd_t = tpool.tile([128, tcn], dt.float32, tag="rstdc")
                        nc.scalar.activation(rstd_t[:], rveps[:], SQT,
                                             bias=zero_col[:])
                        nmr_c = tpool.tile([128, tcn], dt.float32, tag="nmrc")
                        nc.vector.scalar_tensor_tensor(
                            nmr_c[:], mu[:], -1.0, rstd_t[:], AL.mult, AL.mult)

                        # pass B: apply (ACT) + aggregate (PE)
                        for ti in range(tcn):
                            blk, isfirst, islast = blk_of_tile[t0c + ti]
                            act = wpool.tile([128, EMB], dt.bfloat16, tag="act")
                            if fl_triv:
                                nc.scalar.activation(
                                    act[:], xw_c[:, ti, :], LR,
                                    bias=nmr_c[:, ti : ti + 1],
                                    scale=rstd_t[:, ti : ti + 1], alpha=SLOPE)
                            else:
                                y1 = wpool.tile([128, EMB], dt.float32, tag="y1")
                                nc.vector.tensor_scalar(
                                    y1[:], xw_c[:, ti, :], mu[:, ti : ti + 1],
                                    rstd_t[:, ti : ti + 1], AL.subtract, AL.mult)
                                y2 = wpool.tile([128, EMB], dt.float32, tag="y2")
                                nc.vector.scalar_tensor_tensor(
                                    y2[:], y1[:], 1.0, cw[pre + "flg"][:], AL.mult, AL.mult)
                                y3 = wpool.tile([128, EMB], dt.float32, tag="y3")
                                nc.vector.tensor_add(y3[:], y2[:], cw[pre + "flb"][:])
                                nc.scalar.activation(act[:], y3[:], LR,
                                                     bias=zero_col[:], alpha=SLOPE)
                            if cur_ps[0] is None:
                                psb_new = psagg.tile([128, EMB], dt.float32, tag="agg")
                                cur_ps[0] = psb_new
                            psb = cur_ps[0]
                            nc.tensor.matmul(
                                psb[:], ohe[:, ti * 128 : (ti + 1) * 128], act[:],
                                start=isfirst, stop=islast)
                            if islast:
                                nc.vector.tensor_add(acc[:, blk, :], acc[:, blk, :], psb[:])
                                cur_ps[0] = None

                    tdone = 0
                    while tdone < ntiles:
                        # issue gathers in q0/q1 pairs so the two SWDGE queues'
                        # Q7 core pairs generate descriptors concurrently
                        batch = []
                        for _ in range(2):
                            if tdone >= ntiles:
                                break
                            tcn = min(CHUNK_TILES, ntiles - tdone)
                            batch.append((tdone, tcn, fetch(tdone, tcn)))
                            tdone += tcn
                        for (t0c, tcn, (sbuf, ohe, ohT)) in batch:
                            process(t0c, tcn, sbuf, ohe, ohT)

            # =========== post-conv: mean -> wf -> LN -> MLP ===========
            def conv_post(cv, pre, acc, rightT, outT, lpout_name, lpout_w, lpout_b,
                          n_valid, recip_sb, ind_sb):
                nblocks = cv.nblocks
                for g0 in range(0, nblocks, 8):
                    gb = min(8, nblocks - g0)
                    ub = sqpool.tile([128, 8, EMB], dt.float32, tag="pub")
                    for k in range(gb):
                        blk = g0 + k
                        mean = wpool.tile([128, EMB], dt.bfloat16, tag="mean")
                        nc.vector.tensor_scalar_mul(mean[:], acc[:, blk, :],
                                                    recip_sb[:, blk : blk + 1])
                        meanT = wpool.tile([128, 128], dt.bfloat16, tag="meanT")
                        transpose_to(meanT[:], mean[:], 128, 128)
                        ps = psA.tile([128, EMB], dt.float32, tag="ps")
                        nc.tensor.matmul(ps[:], meanT[:], cw[pre + "wf"][:],
                                         start=True, stop=True)
                        nc.vector.scalar_tensor_tensor(
                            ub[:, k, :], cw[pre + "bf"][:], ind_sb[:, blk : blk + 1],
                            ps[:], AL.mult, AL.add)
                    # batched LN stats over the group (plg/plb folded into wo1a/bo1)
                    psx = tpool.tile([128, 8], dt.float32, tag="psx")
                    nc.vector.reduce_sum(psx[:, :gb], ub[:, :gb, :],
                                         axis=mybir.AxisListType.X)
                    sqg = sqpool.tile([128, CHUNK_TILES, EMB], dt.bfloat16, tag="sqc")
                    nc.vector.tensor_tensor(sqg[:, :gb, :], ub[:, :gb, :],
                                            ub[:, :gb, :], AL.mult)
                    psxx = tpool.tile([128, 8], dt.float32, tag="psxx")
                    nc.vector.reduce_sum(psxx[:, :gb], sqg[:, :gb, :],
                                         axis=mybir.AxisListType.X)
                    inv = 1.0 / EMB
                    pmu = tpool.tile([128, 8], dt.float32, tag="pmu")
                    nc.vector.tensor_scalar_mul(pmu[:, :gb], psx[:, :gb], inv)
                    pveps = tpool.tile([128, 8], dt.float32, tag="pveps")
                    nc.vector.tensor_scalar(pveps[:, :gb], psxx[:, :gb], inv, EPS,
                                            AL.mult, AL.add)
                    pnmusq = tpool.tile([128, 8], dt.float32, tag="pnmusq")
                    nc.vector.scalar_tensor_tensor(
                        pnmusq[:, :gb], pmu[:, :gb], -1.0, pmu[:, :gb],
                        AL.mult, AL.mult)
                    nc.vector.tensor_tensor(pveps[:, :gb], pveps[:, :gb],
                                            pnmusq[:, :gb], AL.add)
                    prv = tpool.tile([128, 8], dt.float32, tag="prv")
                    nc.vector.reciprocal(prv[:, :gb], pveps[:, :gb])
                    prstd_t = tpool.tile([128, 8], dt.float32, tag="prstdt")
                    nc.scalar.activation(prstd_t[:, :gb], prv[:, :gb], SQT,
                                         bias=zero_col[:])
                    for k in range(gb):
                        blk = g0 + k
                        lo = blk * 128
                        nv = min(128, n_valid - lo)
                        lnv = wpool.tile([128, EMB], dt.bfloat16, tag="lnv")
                        nc.vector.tensor_scalar(
                            lnv[:], ub[:, k, :], pmu[:, k : k + 1],
                            prstd_t[:, k : k + 1], AL.subtract, AL.mult)
                        lnT = wpool.tile([128, 128], dt.bfloat16, tag="lnT")
                        transpose_to(lnT[:], lnv[:], 128, 128)
                        ps2 = psA.tile([128, EMB], dt.float32, tag="ps")
                        nc.tensor.matmul(ps2[:], lnT[:], cw[pre + "wo1a"][:],
                                         start=True, stop=False)
                        nc.tensor.matmul(ps2[:], rightT[:, lo : lo + 128],
                                         cw[pre + "wo1b"][:], start=False, stop=True)
                        u2 = wpool.tile([128, EMB], dt.float32, tag="pcu2")
                        nc.vector.tensor_add(u2[:], ps2[:], cw[pre + "bo1"][:])
                        tml = wpool.tile([128, EMB], dt.bfloat16, tag="tml")
                        nc.scalar.activation(tml[:], u2[:], LR, bias=zero_col[:],
                                             alpha=SLOPE)
                        tT = wpool.tile([128, 128], dt.bfloat16, tag="tT")
                        transpose_to(tT[:], tml[:], 128, 128)
                        ps3 = psA.tile([128, EMB], dt.float32, tag="ps")
                        nc.tensor.matmul(ps3[:], tT[:], cw[pre + "wo2"][:],
                                         start=True, stop=True)
                        u3 = wpool.tile([128, EMB], dt.float32, tag="pcu3")
                        nc.vector.tensor_add(u3[:], ps3[:], cw[pre + "bo2"][:])
                        res = wpool.tile([128, EMB], dt.bfloat16, tag="res")
                        nc.scalar.copy(res[:], u3[:])
                        transpose_to(outT[:, lo : lo + 128], res[:], 128, 128)
                        if lpout_name is not None and nv > 0:
                            ps4 = psA.tile([128, EMB], dt.float32, tag="ps")
                            nc.tensor.matmul(ps4[:], outT[:, lo : lo + 128],
                                             cw[lpout_w][:], start=True, stop=True)
                            ub4 = wpool.tile([128, EMB], dt.float32, tag="pc4u")
                            nc.vector.tensor_add(ub4[:], ps4[:], cw[lpout_b][:])
                            ob = wpool.tile([128, EMB], dt.bfloat16, tag="pc4o")
                            nc.scalar.copy(ob[:], ub4[:])
                            nc.sync.dma_start(b.d[lpout_name][lo : lo + nv, :],
                                              ob[:nv, :])

            # host-known scatter-mean counts
            rec1 = cpool.tile([128, c1p.nblocks], dt.float32, tag="rec1")
            nc.sync.dma_start(rec1[:], b.d["e1_recip"][:])
            ind1 = cpool.tile([128, c1p.nblocks], dt.float32, tag="ind1")
            nc.sync.dma_start(ind1[:], b.d["e1_ind"][:])
            rec2 = cpool.tile([128, c2p.nblocks], dt.float32, tag="rec2")
            nc.sync.dma_start(rec2[:], b.d["e2_recip"][:])
            ind2 = cpool.tile([128, c2p.nblocks], dt.float32, tag="ind2")
            nc.sync.dma_start(ind2[:], b.d["e2_ind"][:])

            qsel = [0]
            # conv1
            acc1 = rpool.tile([128, c1p.nblocks, EMB], dt.float32, tag="acc1")
            nc.vector.memset(acc1[:], 0.0)
            if KSTAGE not in ("A", "AG1"):
                _lp1src = rp1_loc if KSTAGE == "C1local" else lp1_full
                conv_edges(c1p, "vc_", _lp1src, rp1_loc, b.d["e1_src"],
                           b.d["e1_oh"], b.d["e1_ohT"], acc1, NCL, qsel)
            if KSTAGE not in ("A", "AG1", "C1"):
                conv_post(c1p, "vc_", acc1, c0T, c1T, "lp2_loc", "cv_wl", "cv_bl",
                          NCL, rec1, ind1)
                nc.gpsimd.collective_compute(
                    "AllGather", AL.bypass, ins=[lp2_loc[:]], outs=[lp2_full[:]],
                    replica_groups=[list(range(NCORES))])
            # conv2
            acc2 = rpool.tile([128, c2p.nblocks, EMB], dt.float32, tag="acc2")
            nc.vector.memset(acc2[:], 0.0)
            if KSTAGE not in ("A", "AG1", "C1", "P1"):
                conv_edges(c2p, "cv_", lp2_full, rp2_loc, b.d["e2_src"],
                           b.d["e2_oh"], b.d["e2_ohT"], acc2, NVL, qsel)
                conv_post(c2p, "cv_", acc2, v0T, v1T, None, None, None,
                          NVL, rec2, ind2)

            # =========== heads ===========
            if KSTAGE != "full" or nact == 0:
                zrow = wpool.tile([1, 512], dt.bfloat16, tag="orow")
                nc.vector.memset(zrow[:], 0.0)
                for j in range(NVLh // 512):
                    nc.sync.dma_start(out_d[:, j * 512 : (j + 1) * 512], zrow[:])
            else:
                nch = NVLh // 512
                for j in range(nch):
                    pso = psout.tile([1, 512], dt.float32, tag="pso")
                    for hi in range(nact):
                        ps = psA.tile([128, 512], dt.float32, tag="ps")
                        nc.tensor.matmul(ps[:], cw["hw1"][:, hi, :],
                                         v1T[:, j * 512 : (j + 1) * 512],
                                         start=True, stop=True)
                        hh = wpool.tile([128, 512], dt.bfloat16, tag="hh")
                        nc.scalar.activation(hh[:], ps[:], LR,
                                             bias=cw["hb1"][:, hi : hi + 1],
                                             scale=1.0, alpha=SLOPE)
                        nc.tensor.matmul(pso[:], cw["hw2"][:, hi : hi + 1], hh[:],
                                         start=(hi == 0), stop=(hi == nact - 1))
                    # raw head sum; host applies out_scale/out_add affine
                    orow = cpool.tile([1, 512], dt.bfloat16, tag="orow")
                    nc.scalar.copy(orow[:], pso[:])
                    nc.sync.dma_start(out_d[:, j * 512 : (j + 1) * 512], orow[:])

    nc.compile()
    return b


_CACHE = {}


def kernel(**inputs):
    key = tuple(sorted((k, tuple(np.asarray(v).shape)) for k, v in inputs.items()))
    p = host_prep(inputs)
    ck = (key, p["nact"], p["conv1"].etot, p["conv2"].etot,
          p["vc_fl_trivial"], p["cv_fl_trivial"])
    if ck in _CACHE:
        b = _CACHE[ck]
    else:
        b = build_program(p)
        _CACHE[ck] = b
    in_maps = [dict(p["core_inputs"][c]) for c in range(NCORES)]
    res = run_bass_kernel_spmd(b.nc, in_maps, core_ids=list(range(NCORES)))
    NVL = p["NVL"]
    out = np.concatenate([res.results[c]["out"][0, :NVL] for c in range(NCORES)])
    out = out.astype(np.float32) * p["out_scale"] + p["out_add"]
    return out.astype(np.float32)

